# revision 27
# baseline (speedup 1.0000x reference)
"""Single-head causal self-attention (T=8192, C=1024, fp32) on 8 Trainium2 cores.

Sharding: interleaved over sequence. Core i owns rows {i, i+8, i+16, ...} (T/8
rows) as both queries and keys; causal work is exactly balanced and the
program is SPMD-static.

Per-core pipeline (bf16 matmuls, fp32 PSUM), v2:
  1. K^T and V projections per key segment write into ONE combined DRAM
     buffer per segment, so each segment needs a single AllGather (2 total
     instead of 4) -- the CC stream was the critical path at 4 serialized
     gathers.  Segments are asymmetric ([6, 2] m-blocks): seg0 unblocks
     attention chunks 0-2, and its K blocks are cached in SBUF.
  2. Q projection runs after the segment-1 gather is triggered, hiding both
     AllGathers entirely behind PE work.
  3. Flash-style attention with scores TRANSPOSED (keys on partitions,
     queries on the free axis) -- every matmul transpose-free:
       S^T[j,q] = sum_c KT[c,j] * QT[c,q]         (lhsT=KT block, rhs=QT)
       Y^T[c,q]  = sum_j V[j,c] * P[j,q]          (lhsT=V block,  rhs=P)
     exp() needs no max-subtraction: scores/sqrt(C) ~ N(0,1).  Causal mask
     via 16 precomputed 0/1 comparison tiles on diagonal blocks only;
     fully-masked 64-aligned query spans are skipped.
  4. SOFTWARE PIPELINE on the PE queue: visit v+1's score matmuls are
     emitted BEFORE visit v's AV matmuls.  The PE queue is strict FIFO, so
     the old order (scores_v, AV_v) stalled the PE ~0.5us per visit waiting
     for exp(v); with the interleave exp(v) runs under scores(v+1).
  5. Softmax normalization DEFERRED past the output projection: row sums
     are accumulated on DVE in fp32 ("grand" tile), transposed row sums are
     produced by two N=1 matmuls per chunk, and 1/rowsum is applied as the
     per-partition scalar of the output-projection epilogue
     (scalar_tensor_tensor: (pp * recip) + bias, one DVE op).  This removes
     the reciprocal->partition_broadcast->multiply chain from the PE
     critical path at every chunk boundary, and frees a PSUM bank so the
     output projection is double-buffered.
  6. K blocks of seg0 live in a 96KB/partition SBUF cache (loaded once when
     their AllGather lands, reused by all chunks); V streams per visit.
     This halves steady-state HBM read traffic, which was pacing the PE.
  7. Queues: first wk/xt chunks on gpsimd (earliest-booting engine), bulk
     loads + kt-cache fills + y stores on sync, V-visit loads on gpsimd,
     projection stores on scalar; during attention the scalar queue runs
     exp() only.
"""
import sys

sys.path.insert(0, "/opt/trn_rl_repo")

from contextlib import ExitStack

import numpy as np
import ml_dtypes

import concourse.bacc as bacc
import concourse.mybir as mybir
import concourse.tile as tile
from concourse.bass_utils import run_bass_kernel_spmd

P = 128
NCORES = 8

F32 = mybir.dt.float32
F32R = mybir.dt.float32r
BF16 = mybir.dt.bfloat16
AF = mybir.ActivationFunctionType
ALU = mybir.AluOpType


def build_nc(T, C, debug=False, mock_cc=False, kv_bufs=6, half_diag=True,
             pt_bufs=5, QALIGN=64, qkv_psum_bufs=8, stage_bufs=4,
             st_bufs=8, out_bufs=2, ktb_bufs=4, cache_blocks=6):
    """Build the SPMD program for all 8 cores."""
    NC = NCORES
    R = T // NC          # own rows per core
    DC = C // P          # contraction chunks of 128
    QC = min(2 * P, R)   # query-chunk width (free dim of attention matmuls)
    NCH = R // QC        # query chunks per core
    MBK = R // P         # key m-blocks per rank
    # one collective segment per pair of m-blocks: small early segments keep
    # the serial CC stream (whose wall time scales with TOTAL gathered bytes)
    # fully hidden behind projections + early attention chunks.
    seg_blocks = [min(2, MBK - s) for s in range(0, MBK, 2)]
    seg_start = [0]
    for b in seg_blocks[:-1]:
        seg_start.append(seg_start[-1] + b)
    NSEG = len(seg_blocks)
    CACHE_B = min(cache_blocks, MBK)   # leading m-blocks cached in SBUF
    scale = 1.0 / float(np.sqrt(C))

    nc = bacc.Bacc("TRN2", target_bir_lowering=False, debug=False, num_devices=NC)

    # ---- kernel I/O (per-core data) ----
    xT = nc.dram_tensor("xT", [C, R], BF16, kind="ExternalInput").ap()
    wqT = nc.dram_tensor("wqT", [C, C], BF16, kind="ExternalInput").ap()
    wkT = nc.dram_tensor("wkT", [C, C], BF16, kind="ExternalInput").ap()
    wvT = nc.dram_tensor("wvT", [C, C], BF16, kind="ExternalInput").ap()
    wpT = nc.dram_tensor("wpT", [C, C], BF16, kind="ExternalInput").ap()
    bqT = nc.dram_tensor("bqT", [P, DC], F32, kind="ExternalInput").ap()
    bkT = nc.dram_tensor("bkT", [P, DC], F32, kind="ExternalInput").ap()
    bv = nc.dram_tensor("bv", [1, C], F32, kind="ExternalInput").ap()
    bp = nc.dram_tensor("bp", [1, C], F32, kind="ExternalInput").ap()
    qg = nc.dram_tensor("qg", [1, QC], F32, kind="ExternalInput").ap()   # i + 8f
    pv = nc.dram_tensor("pv", [P, 1], F32, kind="ExternalInput").ap()    # 8p
    y = nc.dram_tensor("y", [R, C], BF16, kind="ExternalOutput").ap()

    with tile.TileContext(nc) as tc, ExitStack() as ctx:
        const = ctx.enter_context(tc.tile_pool(name="const", bufs=1))
        wpool = ctx.enter_context(tc.tile_pool(name="weights", bufs=1))
        dram = ctx.enter_context(tc.tile_pool(name="dram", bufs=1, space="DRAM"))
        stage = ctx.enter_context(tc.tile_pool(name="stage", bufs=stage_bufs))

        # ---- weights & xT in SBUF, parallelized over four DMA queues so
        # the dd-outer K projection can start as soon as wk/xt's dd=0
        # pieces land and then stream: sync=wk,wq  scalar=xt  gpsimd=wv
        # vector=wp. ----
        qkvw_ctx = ExitStack()
        qkvw = qkvw_ctx.enter_context(tc.tile_pool(name="qkvw", bufs=1))

        wk_sb = qkvw.tile([P, DC, C], BF16, tag="wk", name="wk")
        xt_sb = qkvw.tile([P, DC, R], BF16, tag="xt")
        wkT_r = wkT.rearrange("(dd p) c -> p dd c", p=P)
        xT_r = xT.rearrange("(dd p) l -> p dd l", p=P)
        # split the dd=0 pieces so the very first matmul's dependencies are
        # ~160KB instead of 512KB (early DMA runs well below peak rate)
        nc.sync.dma_start(out=wk_sb[:, 0, 0:P], in_=wkT_r[:, 0, 0:P])
        nc.scalar.dma_start(out=xt_sb[:, 0, 0:512], in_=xT_r[:, 0, 0:512])
        nc.sync.dma_start(out=wk_sb[:, 0, P:], in_=wkT_r[:, 0, P:])
        nc.scalar.dma_start(out=xt_sb[:, 0, 512:], in_=xT_r[:, 0, 512:])
        for dd in range(1, DC):
            nc.sync.dma_start(out=wk_sb[:, dd, :], in_=wkT_r[:, dd, :])
            nc.scalar.dma_start(out=xt_sb[:, dd, :], in_=xT_r[:, dd, :])

        def load_pdc(pool, name, src, queue):
            t = pool.tile([P, DC, C], BF16, tag=name, name=name)
            queue.dma_start(
                out=t[:], in_=src.rearrange("(dd p) c -> p dd c", p=P)
            )
            return t

        wv_sb = load_pdc(qkvw, "wv", wvT, nc.gpsimd)
        wq_sb = load_pdc(qkvw, "wq", wqT, nc.sync)
        wp_sb = load_pdc(wpool, "wp", wpT, nc.gpsimd)

        # ---- constants / small inputs (gpsimd queue) ----
        bqT_sb = const.tile([P, DC], F32, tag="bqT")
        bkT_sb = const.tile([P, DC], F32, tag="bkT")
        nc.gpsimd.dma_start(out=bqT_sb[:], in_=bqT[:])
        nc.gpsimd.dma_start(out=bkT_sb[:], in_=bkT[:])
        bv_row = const.tile([1, C], F32, tag="bv_row")
        bp_row = const.tile([1, C], F32, tag="bp_row")
        nc.gpsimd.dma_start(out=bv_row[:], in_=bv[:])
        nc.gpsimd.dma_start(out=bp_row[:], in_=bp[:])
        bv_bc = const.tile([P, C], F32, tag="bv_bc")
        bp_bc = const.tile([P, C], F32, tag="bp_bc")
        nc.gpsimd.partition_broadcast(bv_bc[:], bv_row[:])
        nc.gpsimd.partition_broadcast(bp_bc[:], bp_row[:])
        ones_f32 = const.tile([P, 1], F32, tag="ones")
        nc.vector.memset(ones_f32[:], 1.0)

        # D[p, f] = i + NC*f - NC*p  (per-core causal helper)
        qg_sb = const.tile([1, QC], F32, tag="qg")
        pv_sb = const.tile([P, 1], F32, tag="pv")
        nc.gpsimd.dma_start(out=qg_sb[:], in_=qg[:])
        nc.gpsimd.dma_start(out=pv_sb[:], in_=pv[:])
        qg_bc = const.tile([P, QC], F32, tag="qg_bc")
        nc.gpsimd.partition_broadcast(qg_bc[:], qg_sb[:])
        d_sb = const.tile([P, QC], F32, tag="D")
        nc.vector.tensor_scalar(
            out=d_sb[:], in0=qg_bc[:], scalar1=pv_sb[:], scalar2=None,
            op0=ALU.subtract,
        )

        # ---- internal DRAM: ONE combined K+V buffer per segment, ONE
        # AllGather per segment.  Layout per rank (flat elements):
        #   [0 : P*ktW)        kt:  kt[p, ml*DC*P + cc*P + kk]
        #   [P*ktW : FLAT)     v:   v[ml*P + j, c]
        kv_own, kvg, ktW_s, flat_s = [], [], [], []
        for s in range(NSEG):
            Bs = seg_blocks[s]
            ktW = Bs * DC * P
            FLAT = P * ktW + Bs * P * C
            ktW_s.append(ktW)
            flat_s.append(FLAT)
            kv_own.append(dram.tile([1, FLAT], BF16, name=f"kv_own{s}",
                                    tag=f"kv_own{s}"))
            kvg.append(dram.tile([NC, FLAT], BF16, addr_space="Shared",
                                 name=f"kvg{s}", tag=f"kvg{s}"))
        groups = [list(range(NC))]

        def allgather(in_t, out_t):
            if mock_cc:
                nc.gpsimd.dma_start(out=out_t[0:1, :], in_=in_t[:])
            else:
                nc.gpsimd.collective_compute(
                    "AllGather", ALU.bypass, replica_groups=groups,
                    ins=[in_t.opt()], outs=[out_t.opt()],
                )

        def ktv_view(s):
            return kv_own[s][:, 0:P * ktW_s[s]].rearrange(
                "a (p ml x) -> (a p) ml x", p=P, ml=seg_blocks[s])

        def vv_view(s):
            return kv_own[s][:, P * ktW_s[s]:].rearrange(
                "a (r f) -> (a r) f", r=seg_blocks[s] * P)

        with tc.tile_pool(name="qkv_psum", bufs=qkv_psum_bufs, space="PSUM") as qkv_psum:
            # K^T in 512-row slabs, dd-OUTER (8 open psum banks) so the
            # matmuls stream against the still-loading wk/xt dd pieces.
            # Each slab's stores are split across its segments, and each
            # segment's combined K+V AllGather is issued the moment its
            # last store is queued.  Stores ride the scalar queue;
            # collectives own the gpsimd queue.
            def k_slab(n0, w):
                ps = [qkv_psum.tile([P, 512], F32, tag=f"ps{mc}", bufs=1,
                                    name=f"ps{mc}") for mc in range(DC)]
                for dd in range(DC):
                    for mc in range(DC):
                        nc.tensor.matmul(
                            ps[mc][:, :w],
                            wk_sb[:, dd, mc * P:(mc + 1) * P],
                            xt_sb[:, dd, n0:n0 + w],
                            start=(dd == 0), stop=(dd == DC - 1),
                        )
                for mc in range(DC):
                    st = stage.tile([P, 512], BF16, tag="st", bufs=st_bufs)
                    # alternate eviction engines so the bank frees 2x faster
                    if mc % 2 == 0:
                        nc.scalar.activation(
                            st[:, :w], ps[mc][:, :w], AF.Identity,
                            bias=bkT_sb[:, mc:mc + 1]
                        )
                    else:
                        nc.vector.tensor_scalar(
                            out=st[:, :w], in0=ps[mc][:, :w],
                            scalar1=bkT_sb[:, mc:mc + 1], scalar2=None,
                            op0=ALU.add,
                        )
                    for h0 in range(0, w, 2 * P):
                        hw = min(2 * P, w - h0)
                        k0 = (n0 + h0) // P
                        s = k0 // 2
                        ml0 = k0 - seg_start[s]
                        nm = hw // P
                        nc.sync.dma_start(
                            out=ktv_view(s)[:, ml0:ml0 + nm, mc * P:(mc + 1) * P],
                            in_=st[:, h0:h0 + hw].rearrange(
                                "p (j k) -> p j k", j=nm),
                        )

            def v_block(jb):
                s, ml = jb // 2, jb % 2
                for n in range((C + 511) // 512):
                    ps = qkv_psum.tile([P, 512], F32, tag=f"ps{n % 2}",
                                       bufs=1, name="ps")
                    for dd in range(DC):
                        nc.tensor.matmul(
                            ps[:],
                            xt_sb[:, dd, jb * P:(jb + 1) * P],
                            wv_sb[:, dd, n * 512:(n + 1) * 512],
                            start=(dd == 0), stop=(dd == DC - 1),
                        )
                    st = stage.tile([P, 512], BF16, tag="st", bufs=st_bufs)
                    nc.vector.tensor_add(
                        out=st[:], in0=ps[:], in1=bv_bc[:, n * 512:(n + 1) * 512]
                    )
                    nc.sync.dma_start(
                        out=vv_view(s)[ml * P:(ml + 1) * P,
                                       n * 512:(n + 1) * 512],
                        in_=st[:],
                    )

            # slab 0 -> V0..V3 with AGs for segments 0,1; slab 1 -> V4..V7
            # with AGs for segments 2,3.  (General in MBK via the loop.)
            for half in range(0, MBK, 4):
                w = min(512, R - half * P)
                k_slab(half * P, w)
                for s in range(half // 2, min(half // 2 + 2, NSEG)):
                    for ml in range(seg_blocks[s]):
                        v_block(seg_start[s] + ml)
                    allgather(kv_own[s], kvg[s])

            # Q^T for own query rows (kept in SBUF)
            qt_sb = wpool.tile([P, DC, R], BF16, tag="qt")
            for n0 in range(0, R, 512):
                w = min(512, R - n0)
                ps = [qkv_psum.tile([P, 512], F32, tag=f"ps{mc}", bufs=1,
                                    name=f"ps{mc}") for mc in range(DC)]
                for dd in range(DC):
                    for mc in range(DC):
                        nc.tensor.matmul(
                            ps[mc][:, :w],
                            wq_sb[:, dd, mc * P:(mc + 1) * P],
                            xt_sb[:, dd, n0:n0 + w],
                            start=(dd == 0), stop=(dd == DC - 1),
                        )
                for mc in range(DC):
                    if mc % 2 == 0:
                        nc.scalar.activation(
                            qt_sb[:, mc, n0:n0 + w], ps[mc][:, :w],
                            AF.Identity, bias=bqT_sb[:, mc:mc + 1],
                        )
                    else:
                        nc.vector.tensor_scalar(
                            out=qt_sb[:, mc, n0:n0 + w], in0=ps[mc][:, :w],
                            scalar1=bqT_sb[:, mc:mc + 1], scalar2=None,
                            op0=ALU.add,
                        )

        qkvw_ctx.close()

        # precompute the (few) distinct causal comparison tiles
        cmp_tiles = {}

        def get_cmp(thr):
            if thr not in cmp_tiles:
                t = const.tile([P, QC], BF16, tag=f"cmp{thr}", name=f"cmp{thr}")
                nc.vector.tensor_scalar(
                    out=t[:], in0=d_sb[:], scalar1=float(thr), scalar2=None,
                    op0=ALU.is_ge,
                )
                cmp_tiles[thr] = t
            return cmp_tiles[thr]

        # ---- attention ----
        # leading CACHE_B key m-blocks cached in SBUF (one DMA per block,
        # on first use); later K blocks and all V blocks stream per visit.
        ktc_pool = ctx.enter_context(tc.tile_pool(name="ktc", bufs=1))
        ktc_sb = ktc_pool.tile([P, CACHE_B * NC, DC * P], BF16, tag="ktc")
        ktc_loaded = set()

        kv = ctx.enter_context(tc.tile_pool(name="kv", bufs=kv_bufs))
        ptp = ctx.enter_context(tc.tile_pool(name="pt", bufs=pt_bufs))
        ytp = ctx.enter_context(tc.tile_pool(name="yt", bufs=2))
        gpool = ctx.enter_context(tc.tile_pool(name="gpool", bufs=1))
        smalls = ctx.enter_context(tc.tile_pool(name="smalls", bufs=2))
        s_psum = ctx.enter_context(tc.tile_pool(name="s_psum", bufs=2, space="PSUM"))
        y_psum = ctx.enter_context(tc.tile_pool(name="y_psum", bufs=1, space="PSUM"))
        p_psum = ctx.enter_context(tc.tile_pool(name="p_psum", bufs=2, space="PSUM"))

        grand = gpool.tile([P, QC], F32, tag="grand")

        def seg_of(k):
            for s in range(NSEG - 1, -1, -1):
                if k >= seg_start[s]:
                    return s, k - seg_start[s]
            raise AssertionError

        def kt_gathered(s, ml, r):
            ktW = ktW_s[s]
            return kvg[s][r:r + 1, 0:P * ktW].rearrange(
                "a (p w) -> (a p) w", p=P)[:, ml * DC * P:(ml + 1) * DC * P]

        def v_gathered(s, ml, r):
            off = P * ktW_s[s] + ml * P * C
            return kvg[s][r:r + 1, off:off + P * C].rearrange(
                "a (p f) -> (a p) f", p=P)

        def get_kt(k, r):
            s, ml = seg_of(k)
            if k < CACHE_B:
                b = k * NC + r
                if b not in ktc_loaded:
                    nc.sync.dma_start(out=ktc_sb[:, b, :],
                                      in_=kt_gathered(s, ml, r))
                    ktc_loaded.add(b)
                return ktc_sb[:, b, :]
            t = kv.tile([P, DC * P], BF16, tag="ktb", bufs=ktb_bufs)
            nc.sync.dma_start(out=t[:], in_=kt_gathered(s, ml, r))
            return t

        for c in range(NCH):
            KB = QC * (c + 1) // P  # key m-blocks per rank for this chunk
            n_visits = KB * NC
            y_ps = [y_psum.tile([P, 2 * QC], F32, tag=f"y{t}", name=f"y_ps{t}")
                    for t in range(DC // 2)]
            nc.vector.memset(grand[:], 0.0)

            # prefetch the kt cache blocks this chunk AND the next one will
            # need, so their DMAs queue ahead of this chunk's y stores and
            # fire the moment their segment's AllGather lands.
            KB_next = min(QC * (c + 2) // P, MBK)
            for k in range(min(KB_next, CACHE_B)):
                for r in range(NC):
                    get_kt(k, r)

            yt_sb = ytp.tile([P, DC, QC], BF16, tag="yt")

            # per-qm chunk tail: transposed row sum (one N=1 matmul), its
            # reciprocal, unnormalized Y^T -> bf16, output projection with
            # the 1/rowsum + bias epilogue fused into one DVE op.  qm=0's
            # tail is emitted EARLY (its y columns and row sums are final
            # once block KB-2 completes, since block KB-1 is half-masked),
            # overlapping the output projection with the last visits.
            def emit_qm_tail(qm):
                rs = p_psum.tile([P, 1], F32, tag="pp", name="rs")
                nc.tensor.matmul(rs[:], grand[:, qm * P:(qm + 1) * P],
                                 ones_f32[:], start=True, stop=True)
                recip = smalls.tile([P, 1], F32, tag="recip")
                nc.vector.reciprocal(recip[:], rs[:])
                for cb in range(DC):
                    src = y_ps[cb // 2][:, (cb % 2) * QC + qm * P:
                                        (cb % 2) * QC + (qm + 1) * P]
                    dst = yt_sb[:, cb, qm * P:(qm + 1) * P]
                    if qm == 1 and cb % 2 == 0:
                        nc.scalar.copy(dst, src)
                    else:
                        nc.vector.tensor_copy(out=dst, in_=src)
                for n in range((C + 511) // 512):
                    pp = p_psum.tile([P, 512], F32, tag="pp", name="pp")
                    for cb in range(DC):
                        nc.tensor.matmul(
                            pp[:],
                            yt_sb[:, cb, qm * P:(qm + 1) * P],
                            wp_sb[:, cb, n * 512:(n + 1) * 512],
                            start=(cb == 0), stop=(cb == DC - 1),
                        )
                    out_sb = stage.tile([P, 512], BF16, tag="out", bufs=out_bufs)
                    nc.vector.scalar_tensor_tensor(
                        out=out_sb[:], in0=pp[:], scalar=recip[:],
                        in1=bp_bc[:, n * 512:(n + 1) * 512],
                        op0=ALU.mult, op1=ALU.add,
                    )
                    nc.sync.dma_start(
                        out=y[c * QC + qm * P:c * QC + (qm + 1) * P,
                              n * 512:(n + 1) * 512],
                        in_=out_sb[:],
                    )

            # Software-pipelined visit loop: emit scores(v+1) BEFORE AV(v)
            # so the strict-FIFO PE never waits on exp(v).
            pending = None  # (pt, v_blk, qlo, first, last)

            def emit_av(p):
                pt_t, v_blk, qlo, first, last = p
                for cb in range(DC):
                    # start/stop act on the WHOLE psum bank; only the
                    # first/last write to each bank may carry them.
                    nc.tensor.matmul(
                        y_ps[cb // 2][:, (cb % 2) * QC + qlo:(cb % 2 + 1) * QC],
                        v_blk[:, cb * P:(cb + 1) * P],
                        pt_t[:, qlo:],
                        start=(first and cb % 2 == 0),
                        stop=(last and cb % 2 == 1),
                    )

            vis = 0
            for k in range(KB):
                m0 = k * P
                s, ml = seg_of(k)
                for r in range(NC):
                    first, last = vis == 0, vis == n_visits - 1
                    thr = NC * (m0 - QC * c) + r
                    # queries f < (thr-NC+1)/NC of this tile are fully masked;
                    # skip them (64-aligned) to save PE work.
                    qlo = 0
                    if half_diag and thr > 0:
                        qlo = min(QC, max(0, thr // NC))
                        qlo = (qlo // QALIGN) * QALIGN
                    kt_blk = get_kt(k, r)
                    v_blk = kv.tile([P, C], BF16, tag="vb")
                    # scalar queue: the gpsimd queue is dedicated to the
                    # collectives, which chain back-to-back there and would
                    # block anything queued behind them until the last one.
                    nc.scalar.dma_start(out=v_blk[:], in_=v_gathered(s, ml, r))
                    s_ps = s_psum.tile([P, QC], F32, tag="s")
                    for cc in range(DC):
                        nc.tensor.matmul(
                            s_ps[:, qlo:],
                            kt_blk[:, cc * P:(cc + 1) * P],
                            qt_sb[:, cc, c * QC + qlo:(c + 1) * QC],
                            start=(cc == 0), stop=(cc == DC - 1),
                        )
                    pt = ptp.tile([P, QC], BF16, tag="pt")
                    nc.scalar.activation(pt[:, qlo:], s_ps[:, qlo:], AF.Exp,
                                         scale=scale)
                    if thr > -NC * (P - 1):
                        nc.vector.tensor_mul(
                            out=pt[:, qlo:], in0=pt[:, qlo:],
                            in1=get_cmp(thr)[:, qlo:]
                        )
                    nc.vector.tensor_add(
                        out=grand[:, qlo:], in0=grand[:, qlo:], in1=pt[:, qlo:]
                    )
                    if pending is not None:
                        emit_av(pending)
                    pending = (pt, v_blk, qlo, first, last)
                    vis += 1
                    if QC == 2 * P and half_diag and k == KB - 1 and r == 0:
                        emit_qm_tail(0)
            emit_av(pending)
            if QC == 2 * P and half_diag:
                emit_qm_tail(1)
            else:
                for qm in range(QC // P):
                    emit_qm_tail(qm)

    nc.finalize()
    return nc


_NC_CACHE = {}


TUNED = dict(kv_bufs=8, half_diag=True, pt_bufs=5, qkv_psum_bufs=8,
             st_bufs=8, out_bufs=4, ktb_bufs=4, cache_blocks=6)


def _get_nc(T, C, debug=False):
    key = (T, C, debug)
    if key not in _NC_CACHE:
        kwargs = TUNED if T >= 2048 else {}
        _NC_CACHE[key] = build_nc(T, C, debug, **kwargs)
    return _NC_CACHE[key]


def build_in_maps(inputs):
    x = np.asarray(inputs["x"], dtype=np.float32)
    T, C = x.shape
    NC = NCORES
    DC = C // P
    QC = min(2 * P, T // NC)
    bf = ml_dtypes.bfloat16

    def prep_w(W):
        return np.ascontiguousarray(np.asarray(W, np.float32).T).astype(bf)

    wqT, wkT = prep_w(inputs["Wq"]), prep_w(inputs["Wk"])
    wvT, wpT = prep_w(inputs["Wv"]), prep_w(inputs["Wp"])
    bqT = np.ascontiguousarray(np.asarray(inputs["bq"], np.float32).reshape(DC, P).T)
    bkT = np.ascontiguousarray(np.asarray(inputs["bk"], np.float32).reshape(DC, P).T)
    bv_r = np.asarray(inputs["bv"], np.float32).reshape(1, C)
    bp_r = np.asarray(inputs["bp"], np.float32).reshape(1, C)
    pv = (NC * np.arange(P, dtype=np.float32)).reshape(P, 1)

    in_maps = []
    for i in range(NC):
        xTi = np.ascontiguousarray(x[i::NC].T).astype(bf)
        qg = (i + NC * np.arange(QC, dtype=np.float32)).reshape(1, QC)
        in_maps.append({
            "xT": xTi, "wqT": wqT, "wkT": wkT, "wvT": wvT, "wpT": wpT,
            "bqT": bqT, "bkT": bkT, "bv": bv_r, "bp": bp_r,
            "qg": qg, "pv": pv,
        })
    return in_maps


def kernel(x, Wq, bq, Wk, bk, Wv, bv, Wp, bp, _debug=False, _raw=False):
    x = np.asarray(x, dtype=np.float32)
    T, C = x.shape
    NC = NCORES

    nc = _get_nc(T, C, _debug)
    in_maps = build_in_maps(dict(x=x, Wq=Wq, bq=bq, Wk=Wk, bk=bk,
                                 Wv=Wv, bv=bv, Wp=Wp, bp=bp))

    if _raw or _debug:
        res = run_bass_kernel_spmd(nc, in_maps, list(range(NC)))
        if _raw:
            return res
        results = res.results
    else:
        try:
            results = _run_cached(nc, T, C, in_maps)
        except Exception:
            res = run_bass_kernel_spmd(nc, in_maps, list(range(NC)))
            results = res.results
    y = np.empty((T, C), np.float32)
    for i in range(NC):
        y[i::NC] = np.asarray(results[i]["y"]).astype(np.float32)
    return y


_RUNNER_CACHE = {}


def _run_cached(nc, T, C, in_maps):
    """Repeat-call fast path: the sharded PJRT executable and the device-side
    zero output buffers are built once; later calls only transfer inputs."""
    import jax
    from jax.sharding import Mesh, PartitionSpec, NamedSharding
    from jax.experimental.shard_map import shard_map
    import concourse.bass2jax as b2j
    import concourse.mybir as mb

    key = (T, C)
    if key not in _RUNNER_CACHE:
        b2j.install_neuronx_cc_hook()
        partition_name = (nc.partition_id_tensor.name
                          if nc.partition_id_tensor else None)
        in_names, out_names, out_avals, zero_outs = [], [], [], []
        for alloc in nc.m.functions[0].allocations:
            if not isinstance(alloc, mb.MemoryLocationSet):
                continue
            name = alloc.memorylocations[0].name
            if alloc.kind == "ExternalInput":
                if name != partition_name:
                    in_names.append(name)
            elif alloc.kind == "ExternalOutput":
                shape = tuple(alloc.tensor_shape)
                dtype = mb.dt.np(alloc.dtype)
                out_names.append(name)
                out_avals.append(jax.core.ShapedArray(shape, dtype))
                zero_outs.append(np.zeros(shape, dtype))
        n_params = len(in_names)
        all_in = in_names + out_names + ([partition_name] if partition_name else [])

        def _body(*args):
            operands = list(args)
            if partition_name is not None:
                operands.append(b2j.partition_id_tensor())
            return tuple(b2j._bass_exec_p.bind(
                *operands,
                out_avals=tuple(out_avals),
                in_names=tuple(all_in),
                out_names=tuple(out_names),
                lowering_input_output_aliases=(),
                sim_require_finite=True,
                sim_require_nnan=True,
                nc=nc,
            ))

        devices = jax.devices()[:NCORES]
        mesh = Mesh(np.asarray(devices), ("core",))
        n_outs = len(out_names)
        fn = jax.jit(
            shard_map(_body, mesh=mesh,
                      in_specs=(PartitionSpec("core"),) * (n_params + n_outs),
                      out_specs=(PartitionSpec("core"),) * n_outs,
                      check_rep=False),
            keep_unused=True,
        )
        sharding = NamedSharding(mesh, PartitionSpec("core"))
        zeros_dev = [
            jax.device_put(np.zeros((NCORES * z.shape[0], *z.shape[1:]), z.dtype),
                           sharding)
            for z in zero_outs
        ]
        _RUNNER_CACHE[key] = (fn, in_names, out_names, out_avals, zeros_dev, sharding)

    fn, in_names, out_names, out_avals, zeros_dev, sharding = _RUNNER_CACHE[key]
    import jax
    concat_in = [
        jax.device_put(
            np.concatenate([np.asarray(in_maps[c][n]) for c in range(NCORES)],
                           axis=0), sharding)
        for n in in_names
    ]
    outs = fn(*concat_in, *zeros_dev)
    results = []
    for c in range(NCORES):
        results.append({
            name: np.asarray(outs[i]).reshape(NCORES, *out_avals[i].shape)[c]
            for i, name in enumerate(out_names)
        })
    return results


# revision 29
# speedup vs baseline: 1.0161x; 1.0161x over previous
"""Single-head causal self-attention (T=8192, C=1024, fp32) on 8 Trainium2 cores.

Sharding: interleaved over sequence. Core i owns rows {i, i+8, i+16, ...} (T/8
rows) as both queries and keys; causal work is exactly balanced and the
program is SPMD-static.

Per-core pipeline (bf16 matmuls, fp32 PSUM), v2:
  1. K^T and V projections per key segment write into ONE combined DRAM
     buffer per segment, so each segment needs a single AllGather (2 total
     instead of 4) -- the CC stream was the critical path at 4 serialized
     gathers.  Segments are asymmetric ([6, 2] m-blocks): seg0 unblocks
     attention chunks 0-2, and its K blocks are cached in SBUF.
  2. Q projection runs after the segment-1 gather is triggered, hiding both
     AllGathers entirely behind PE work.
  3. Flash-style attention with scores TRANSPOSED (keys on partitions,
     queries on the free axis) -- every matmul transpose-free:
       S^T[j,q] = sum_c KT[c,j] * QT[c,q]         (lhsT=KT block, rhs=QT)
       Y^T[c,q]  = sum_j V[j,c] * P[j,q]          (lhsT=V block,  rhs=P)
     exp() needs no max-subtraction: scores/sqrt(C) ~ N(0,1).  Causal mask
     via 16 precomputed 0/1 comparison tiles on diagonal blocks only;
     fully-masked 64-aligned query spans are skipped.
  4. SOFTWARE PIPELINE on the PE queue: visit v+1's score matmuls are
     emitted BEFORE visit v's AV matmuls.  The PE queue is strict FIFO, so
     the old order (scores_v, AV_v) stalled the PE ~0.5us per visit waiting
     for exp(v); with the interleave exp(v) runs under scores(v+1).
  5. Softmax normalization DEFERRED past the output projection: row sums
     are accumulated on DVE in fp32 ("grand" tile), transposed row sums are
     produced by two N=1 matmuls per chunk, and 1/rowsum is applied as the
     per-partition scalar of the output-projection epilogue
     (scalar_tensor_tensor: (pp * recip) + bias, one DVE op).  This removes
     the reciprocal->partition_broadcast->multiply chain from the PE
     critical path at every chunk boundary, and frees a PSUM bank so the
     output projection is double-buffered.
  6. K blocks of seg0 live in a 96KB/partition SBUF cache (loaded once when
     their AllGather lands, reused by all chunks); V streams per visit.
     This halves steady-state HBM read traffic, which was pacing the PE.
  7. Queues: first wk/xt chunks on gpsimd (earliest-booting engine), bulk
     loads + kt-cache fills + y stores on sync, V-visit loads on gpsimd,
     projection stores on scalar; during attention the scalar queue runs
     exp() only.
"""
import sys

sys.path.insert(0, "/opt/trn_rl_repo")

from contextlib import ExitStack

import numpy as np
import ml_dtypes

import concourse.bacc as bacc
import concourse.mybir as mybir
import concourse.tile as tile
from concourse.bass_utils import run_bass_kernel_spmd

P = 128
NCORES = 8

F32 = mybir.dt.float32
F32R = mybir.dt.float32r
BF16 = mybir.dt.bfloat16
AF = mybir.ActivationFunctionType
ALU = mybir.AluOpType


def build_nc(T, C, debug=False, mock_cc=False, kv_bufs=6, half_diag=True,
             pt_bufs=5, QALIGN=64, qkv_psum_bufs=8, stage_bufs=4,
             st_bufs=8, out_bufs=2, ktb_bufs=4, cache_blocks=6):
    """Build the SPMD program for all 8 cores."""
    NC = NCORES
    R = T // NC          # own rows per core
    DC = C // P          # contraction chunks of 128
    QC = min(2 * P, R)   # query-chunk width (free dim of attention matmuls)
    NCH = R // QC        # query chunks per core
    MBK = R // P         # key m-blocks per rank
    # one collective segment per pair of m-blocks: small early segments keep
    # the serial CC stream (whose wall time scales with TOTAL gathered bytes)
    # fully hidden behind projections + early attention chunks.
    seg_blocks = [min(2, MBK - s) for s in range(0, MBK, 2)]
    seg_start = [0]
    for b in seg_blocks[:-1]:
        seg_start.append(seg_start[-1] + b)
    NSEG = len(seg_blocks)
    CACHE_B = min(cache_blocks, MBK)   # leading m-blocks cached in SBUF
    scale = 1.0 / float(np.sqrt(C))

    nc = bacc.Bacc("TRN2", target_bir_lowering=False, debug=False, num_devices=NC)

    # ---- kernel I/O (per-core data) ----
    xT = nc.dram_tensor("xT", [C, R], BF16, kind="ExternalInput").ap()
    wqT = nc.dram_tensor("wqT", [C, C], BF16, kind="ExternalInput").ap()
    wkT = nc.dram_tensor("wkT", [C, C], BF16, kind="ExternalInput").ap()
    wvT = nc.dram_tensor("wvT", [C, C], BF16, kind="ExternalInput").ap()
    wpT = nc.dram_tensor("wpT", [C, C], BF16, kind="ExternalInput").ap()
    bqT = nc.dram_tensor("bqT", [P, DC], F32, kind="ExternalInput").ap()
    bkT = nc.dram_tensor("bkT", [P, DC], F32, kind="ExternalInput").ap()
    bv = nc.dram_tensor("bv", [1, C], F32, kind="ExternalInput").ap()
    bp = nc.dram_tensor("bp", [1, C], F32, kind="ExternalInput").ap()
    qg = nc.dram_tensor("qg", [1, QC], F32, kind="ExternalInput").ap()   # i + 8f
    pv = nc.dram_tensor("pv", [P, 1], F32, kind="ExternalInput").ap()    # 8p
    y = nc.dram_tensor("y", [R, C], BF16, kind="ExternalOutput").ap()

    with tile.TileContext(nc) as tc, ExitStack() as ctx:
        const = ctx.enter_context(tc.tile_pool(name="const", bufs=1))
        wpool = ctx.enter_context(tc.tile_pool(name="weights", bufs=1))
        dram = ctx.enter_context(tc.tile_pool(name="dram", bufs=1, space="DRAM"))
        stage = ctx.enter_context(tc.tile_pool(name="stage", bufs=stage_bufs))

        # ---- weights & xT in SBUF, parallelized over four DMA queues so
        # the dd-outer K projection can start as soon as wk/xt's dd=0
        # pieces land and then stream: sync=wk,wq  scalar=xt  gpsimd=wv
        # vector=wp. ----
        qkvw_ctx = ExitStack()
        qkvw = qkvw_ctx.enter_context(tc.tile_pool(name="qkvw", bufs=1))

        wk_sb = qkvw.tile([P, DC, C], BF16, tag="wk", name="wk")
        xt_sb = qkvw.tile([P, DC, R], BF16, tag="xt")
        wkT_r = wkT.rearrange("(dd p) c -> p dd c", p=P)
        xT_r = xT.rearrange("(dd p) l -> p dd l", p=P)
        # split the dd=0 pieces so the very first matmul's dependencies are
        # ~160KB instead of 512KB (early DMA runs well below peak rate)
        nc.sync.dma_start(out=wk_sb[:, 0, 0:P], in_=wkT_r[:, 0, 0:P])
        nc.scalar.dma_start(out=xt_sb[:, 0, 0:512], in_=xT_r[:, 0, 0:512])
        nc.sync.dma_start(out=wk_sb[:, 0, P:], in_=wkT_r[:, 0, P:])
        nc.scalar.dma_start(out=xt_sb[:, 0, 512:], in_=xT_r[:, 0, 512:])
        for dd in range(1, DC):
            nc.sync.dma_start(out=wk_sb[:, dd, :], in_=wkT_r[:, dd, :])
            nc.scalar.dma_start(out=xt_sb[:, dd, :], in_=xT_r[:, dd, :])

        def load_pdc(pool, name, src, queue):
            t = pool.tile([P, DC, C], BF16, tag=name, name=name)
            queue.dma_start(
                out=t[:], in_=src.rearrange("(dd p) c -> p dd c", p=P)
            )
            return t

        wv_sb = load_pdc(qkvw, "wv", wvT, nc.gpsimd)
        wq_sb = load_pdc(qkvw, "wq", wqT, nc.sync)
        wp_sb = load_pdc(wpool, "wp", wpT, nc.gpsimd)

        # ---- constants / small inputs (gpsimd queue) ----
        bqT_sb = const.tile([P, DC], F32, tag="bqT")
        bkT_sb = const.tile([P, DC], F32, tag="bkT")
        nc.gpsimd.dma_start(out=bqT_sb[:], in_=bqT[:])
        nc.gpsimd.dma_start(out=bkT_sb[:], in_=bkT[:])
        bv_row = const.tile([1, C], F32, tag="bv_row")
        bp_row = const.tile([1, C], F32, tag="bp_row")
        nc.gpsimd.dma_start(out=bv_row[:], in_=bv[:])
        nc.gpsimd.dma_start(out=bp_row[:], in_=bp[:])
        bv_bc = const.tile([P, C], F32, tag="bv_bc")
        bp_bc = const.tile([P, C], F32, tag="bp_bc")
        nc.gpsimd.partition_broadcast(bv_bc[:], bv_row[:])
        nc.gpsimd.partition_broadcast(bp_bc[:], bp_row[:])
        ones_f32 = const.tile([P, 1], F32, tag="ones")
        nc.vector.memset(ones_f32[:], 1.0)

        # D[p, f] = i + NC*f - NC*p  (per-core causal helper)
        qg_sb = const.tile([1, QC], F32, tag="qg")
        pv_sb = const.tile([P, 1], F32, tag="pv")
        nc.gpsimd.dma_start(out=qg_sb[:], in_=qg[:])
        nc.gpsimd.dma_start(out=pv_sb[:], in_=pv[:])
        qg_bc = const.tile([P, QC], F32, tag="qg_bc")
        nc.gpsimd.partition_broadcast(qg_bc[:], qg_sb[:])
        d_sb = const.tile([P, QC], F32, tag="D")
        nc.vector.tensor_scalar(
            out=d_sb[:], in0=qg_bc[:], scalar1=pv_sb[:], scalar2=None,
            op0=ALU.subtract,
        )

        # ---- internal DRAM: ONE combined K+V buffer per segment, ONE
        # AllGather per segment.  Layout per rank (flat elements):
        #   [0 : P*ktW)        kt:  kt[p, ml*DC*P + cc*P + kk]
        #   [P*ktW : FLAT)     v:   v[ml*P + j, c]
        kv_own, kvg, ktW_s, flat_s = [], [], [], []
        for s in range(NSEG):
            Bs = seg_blocks[s]
            ktW = Bs * DC * P
            FLAT = P * ktW + Bs * P * C
            ktW_s.append(ktW)
            flat_s.append(FLAT)
            kv_own.append(dram.tile([1, FLAT], BF16, name=f"kv_own{s}",
                                    tag=f"kv_own{s}"))
            kvg.append(dram.tile([NC, FLAT], BF16, addr_space="Shared",
                                 name=f"kvg{s}", tag=f"kvg{s}"))
        groups = [list(range(NC))]

        def allgather(in_t, out_t):
            if mock_cc:
                nc.gpsimd.dma_start(out=out_t[0:1, :], in_=in_t[:])
            else:
                nc.gpsimd.collective_compute(
                    "AllGather", ALU.bypass, replica_groups=groups,
                    ins=[in_t.opt()], outs=[out_t.opt()],
                )

        def ktv_view(s):
            return kv_own[s][:, 0:P * ktW_s[s]].rearrange(
                "a (p ml x) -> (a p) ml x", p=P, ml=seg_blocks[s])

        def vv_view(s):
            return kv_own[s][:, P * ktW_s[s]:].rearrange(
                "a (r f) -> (a r) f", r=seg_blocks[s] * P)

        with tc.tile_pool(name="qkv_psum", bufs=qkv_psum_bufs, space="PSUM") as qkv_psum:
            # K^T one SEGMENT at a time (dd-OUTER over open psum banks, so
            # the matmuls stream against the still-loading wk/xt pieces),
            # immediately followed by that segment's V blocks and its
            # combined K+V AllGather.  Small per-segment units ring each
            # collective's doorbell as early as possible; the collectives
            # then chain back-to-back on the CC stream.  Stores ride sync;
            # the gpsimd queue carries ONLY the collectives (each blocks
            # the issuing queue until it completes).
            def k_seg(s):
                Bs = seg_blocks[s]
                w = Bs * P
                n0 = seg_start[s] * P
                ps = [qkv_psum.tile([P, 512], F32, tag=f"ps{mc}", bufs=1,
                                    name=f"ps{mc}") for mc in range(DC)]
                for dd in range(DC):
                    for mc in range(DC):
                        nc.tensor.matmul(
                            ps[mc][:, :w],
                            wk_sb[:, dd, mc * P:(mc + 1) * P],
                            xt_sb[:, dd, n0:n0 + w],
                            start=(dd == 0), stop=(dd == DC - 1),
                        )
                for mc in range(DC):
                    st = stage.tile([P, 512], BF16, tag="st", bufs=st_bufs)
                    # alternate eviction engines so the banks free 2x faster
                    if mc % 2 == 0:
                        nc.scalar.activation(
                            st[:, :w], ps[mc][:, :w], AF.Identity,
                            bias=bkT_sb[:, mc:mc + 1]
                        )
                    else:
                        nc.vector.tensor_scalar(
                            out=st[:, :w], in0=ps[mc][:, :w],
                            scalar1=bkT_sb[:, mc:mc + 1], scalar2=None,
                            op0=ALU.add,
                        )
                    nc.sync.dma_start(
                        out=ktv_view(s)[:, 0:Bs, mc * P:(mc + 1) * P],
                        in_=st[:, :w].rearrange("p (j k) -> p j k", j=Bs),
                    )

            def v_block(jb):
                s, ml = jb // 2, jb % 2
                for n in range((C + 511) // 512):
                    ps = qkv_psum.tile([P, 512], F32, tag=f"ps{n % 2}",
                                       bufs=1, name="ps")
                    for dd in range(DC):
                        nc.tensor.matmul(
                            ps[:],
                            xt_sb[:, dd, jb * P:(jb + 1) * P],
                            wv_sb[:, dd, n * 512:(n + 1) * 512],
                            start=(dd == 0), stop=(dd == DC - 1),
                        )
                    st = stage.tile([P, 512], BF16, tag="st", bufs=st_bufs)
                    nc.vector.tensor_add(
                        out=st[:], in0=ps[:], in1=bv_bc[:, n * 512:(n + 1) * 512]
                    )
                    nc.sync.dma_start(
                        out=vv_view(s)[ml * P:(ml + 1) * P,
                                       n * 512:(n + 1) * 512],
                        in_=st[:],
                    )

            for s in range(NSEG):
                k_seg(s)
                for ml in range(seg_blocks[s]):
                    v_block(seg_start[s] + ml)
                allgather(kv_own[s], kvg[s])

            # Q^T for own query rows (kept in SBUF)
            qt_sb = wpool.tile([P, DC, R], BF16, tag="qt")
            for n0 in range(0, R, 512):
                w = min(512, R - n0)
                ps = [qkv_psum.tile([P, 512], F32, tag=f"ps{mc}", bufs=1,
                                    name=f"ps{mc}") for mc in range(DC)]
                for dd in range(DC):
                    for mc in range(DC):
                        nc.tensor.matmul(
                            ps[mc][:, :w],
                            wq_sb[:, dd, mc * P:(mc + 1) * P],
                            xt_sb[:, dd, n0:n0 + w],
                            start=(dd == 0), stop=(dd == DC - 1),
                        )
                for mc in range(DC):
                    if mc % 2 == 0:
                        nc.scalar.activation(
                            qt_sb[:, mc, n0:n0 + w], ps[mc][:, :w],
                            AF.Identity, bias=bqT_sb[:, mc:mc + 1],
                        )
                    else:
                        nc.vector.tensor_scalar(
                            out=qt_sb[:, mc, n0:n0 + w], in0=ps[mc][:, :w],
                            scalar1=bqT_sb[:, mc:mc + 1], scalar2=None,
                            op0=ALU.add,
                        )

        qkvw_ctx.close()

        # precompute the (few) distinct causal comparison tiles
        cmp_tiles = {}

        def get_cmp(thr):
            if thr not in cmp_tiles:
                t = const.tile([P, QC], BF16, tag=f"cmp{thr}", name=f"cmp{thr}")
                nc.vector.tensor_scalar(
                    out=t[:], in0=d_sb[:], scalar1=float(thr), scalar2=None,
                    op0=ALU.is_ge,
                )
                cmp_tiles[thr] = t
            return cmp_tiles[thr]

        # ---- attention ----
        # leading CACHE_B key m-blocks cached in SBUF (one DMA per block,
        # on first use); later K blocks and all V blocks stream per visit.
        ktc_pool = ctx.enter_context(tc.tile_pool(name="ktc", bufs=1))
        ktc_sb = ktc_pool.tile([P, CACHE_B * NC, DC * P], BF16, tag="ktc")
        ktc_loaded = set()

        kv = ctx.enter_context(tc.tile_pool(name="kv", bufs=kv_bufs))
        ptp = ctx.enter_context(tc.tile_pool(name="pt", bufs=pt_bufs))
        ytp = ctx.enter_context(tc.tile_pool(name="yt", bufs=2))
        gpool = ctx.enter_context(tc.tile_pool(name="gpool", bufs=1))
        smalls = ctx.enter_context(tc.tile_pool(name="smalls", bufs=2))
        s_psum = ctx.enter_context(tc.tile_pool(name="s_psum", bufs=2, space="PSUM"))
        y_psum = ctx.enter_context(tc.tile_pool(name="y_psum", bufs=1, space="PSUM"))
        p_psum = ctx.enter_context(tc.tile_pool(name="p_psum", bufs=2, space="PSUM"))

        grand = gpool.tile([P, QC], F32, tag="grand")

        def seg_of(k):
            for s in range(NSEG - 1, -1, -1):
                if k >= seg_start[s]:
                    return s, k - seg_start[s]
            raise AssertionError

        def kt_gathered(s, ml, r):
            ktW = ktW_s[s]
            return kvg[s][r:r + 1, 0:P * ktW].rearrange(
                "a (p w) -> (a p) w", p=P)[:, ml * DC * P:(ml + 1) * DC * P]

        def v_gathered(s, ml, r):
            off = P * ktW_s[s] + ml * P * C
            return kvg[s][r:r + 1, off:off + P * C].rearrange(
                "a (p f) -> (a p) f", p=P)

        def get_kt(k, r):
            s, ml = seg_of(k)
            if k < CACHE_B:
                b = k * NC + r
                if b not in ktc_loaded:
                    nc.sync.dma_start(out=ktc_sb[:, b, :],
                                      in_=kt_gathered(s, ml, r))
                    ktc_loaded.add(b)
                return ktc_sb[:, b, :]
            t = kv.tile([P, DC * P], BF16, tag="ktb", bufs=ktb_bufs)
            nc.sync.dma_start(out=t[:], in_=kt_gathered(s, ml, r))
            return t

        for c in range(NCH):
            KB = QC * (c + 1) // P  # key m-blocks per rank for this chunk
            n_visits = KB * NC
            y_ps = [y_psum.tile([P, 2 * QC], F32, tag=f"y{t}", name=f"y_ps{t}")
                    for t in range(DC // 2)]
            nc.vector.memset(grand[:], 0.0)

            # prefetch the kt cache blocks this chunk AND the next one will
            # need, so their DMAs queue ahead of this chunk's y stores and
            # fire the moment their segment's AllGather lands.
            KB_next = min(QC * (c + 2) // P, MBK)
            for k in range(min(KB_next, CACHE_B)):
                for r in range(NC):
                    get_kt(k, r)

            yt_sb = ytp.tile([P, DC, QC], BF16, tag="yt")

            # per-qm chunk tail: transposed row sum (one N=1 matmul), its
            # reciprocal, unnormalized Y^T -> bf16, output projection with
            # the 1/rowsum + bias epilogue fused into one DVE op.  qm=0's
            # tail is emitted EARLY (its y columns and row sums are final
            # once block KB-2 completes, since block KB-1 is half-masked),
            # overlapping the output projection with the last visits.
            def emit_qm_tail(qm):
                rs = p_psum.tile([P, 1], F32, tag="pp", name="rs")
                nc.tensor.matmul(rs[:], grand[:, qm * P:(qm + 1) * P],
                                 ones_f32[:], start=True, stop=True)
                recip = smalls.tile([P, 1], F32, tag="recip")
                nc.vector.reciprocal(recip[:], rs[:])
                for cb in range(DC):
                    src = y_ps[cb // 2][:, (cb % 2) * QC + qm * P:
                                        (cb % 2) * QC + (qm + 1) * P]
                    dst = yt_sb[:, cb, qm * P:(qm + 1) * P]
                    if qm == 1 and cb % 2 == 0:
                        nc.scalar.copy(dst, src)
                    else:
                        nc.vector.tensor_copy(out=dst, in_=src)
                for n in range((C + 511) // 512):
                    pp = p_psum.tile([P, 512], F32, tag="pp", name="pp")
                    for cb in range(DC):
                        nc.tensor.matmul(
                            pp[:],
                            yt_sb[:, cb, qm * P:(qm + 1) * P],
                            wp_sb[:, cb, n * 512:(n + 1) * 512],
                            start=(cb == 0), stop=(cb == DC - 1),
                        )
                    out_sb = stage.tile([P, 512], BF16, tag="out", bufs=out_bufs)
                    nc.vector.scalar_tensor_tensor(
                        out=out_sb[:], in0=pp[:], scalar=recip[:],
                        in1=bp_bc[:, n * 512:(n + 1) * 512],
                        op0=ALU.mult, op1=ALU.add,
                    )
                    nc.sync.dma_start(
                        out=y[c * QC + qm * P:c * QC + (qm + 1) * P,
                              n * 512:(n + 1) * 512],
                        in_=out_sb[:],
                    )

            # Software-pipelined visit loop: emit scores(v+1) BEFORE AV(v)
            # so the strict-FIFO PE never waits on exp(v).
            pending = None  # (pt, v_blk, qlo, first, last)

            def emit_av(p):
                pt_t, v_blk, qlo, first, last = p
                for cb in range(DC):
                    # start/stop act on the WHOLE psum bank; only the
                    # first/last write to each bank may carry them.
                    nc.tensor.matmul(
                        y_ps[cb // 2][:, (cb % 2) * QC + qlo:(cb % 2 + 1) * QC],
                        v_blk[:, cb * P:(cb + 1) * P],
                        pt_t[:, qlo:],
                        start=(first and cb % 2 == 0),
                        stop=(last and cb % 2 == 1),
                    )

            vis = 0
            for k in range(KB):
                m0 = k * P
                s, ml = seg_of(k)
                for r in range(NC):
                    first, last = vis == 0, vis == n_visits - 1
                    thr = NC * (m0 - QC * c) + r
                    # queries f < (thr-NC+1)/NC of this tile are fully masked;
                    # skip them (64-aligned) to save PE work.
                    qlo = 0
                    if half_diag and thr > 0:
                        qlo = min(QC, max(0, thr // NC))
                        qlo = (qlo // QALIGN) * QALIGN
                    kt_blk = get_kt(k, r)
                    v_blk = kv.tile([P, C], BF16, tag="vb")
                    # scalar queue: the gpsimd queue is dedicated to the
                    # collectives, which chain back-to-back there and would
                    # block anything queued behind them until the last one.
                    nc.scalar.dma_start(out=v_blk[:], in_=v_gathered(s, ml, r))
                    s_ps = s_psum.tile([P, QC], F32, tag="s")
                    for cc in range(DC):
                        nc.tensor.matmul(
                            s_ps[:, qlo:],
                            kt_blk[:, cc * P:(cc + 1) * P],
                            qt_sb[:, cc, c * QC + qlo:(c + 1) * QC],
                            start=(cc == 0), stop=(cc == DC - 1),
                        )
                    pt = ptp.tile([P, QC], BF16, tag="pt")
                    nc.scalar.activation(pt[:, qlo:], s_ps[:, qlo:], AF.Exp,
                                         scale=scale)
                    if thr > -NC * (P - 1):
                        nc.vector.tensor_mul(
                            out=pt[:, qlo:], in0=pt[:, qlo:],
                            in1=get_cmp(thr)[:, qlo:]
                        )
                    nc.vector.tensor_add(
                        out=grand[:, qlo:], in0=grand[:, qlo:], in1=pt[:, qlo:]
                    )
                    if pending is not None:
                        emit_av(pending)
                    pending = (pt, v_blk, qlo, first, last)
                    vis += 1
                    if QC == 2 * P and half_diag and k == KB - 1 and r == 0:
                        emit_qm_tail(0)
            emit_av(pending)
            if QC == 2 * P and half_diag:
                emit_qm_tail(1)
            else:
                for qm in range(QC // P):
                    emit_qm_tail(qm)

    nc.finalize()
    return nc


_NC_CACHE = {}


TUNED = dict(kv_bufs=8, half_diag=True, pt_bufs=5, qkv_psum_bufs=8,
             st_bufs=12, out_bufs=4, ktb_bufs=4, cache_blocks=6)


def _get_nc(T, C, debug=False):
    key = (T, C, debug)
    if key not in _NC_CACHE:
        kwargs = TUNED if T >= 2048 else {}
        _NC_CACHE[key] = build_nc(T, C, debug, **kwargs)
    return _NC_CACHE[key]


def build_in_maps(inputs):
    x = np.asarray(inputs["x"], dtype=np.float32)
    T, C = x.shape
    NC = NCORES
    DC = C // P
    QC = min(2 * P, T // NC)
    bf = ml_dtypes.bfloat16

    def prep_w(W):
        return np.ascontiguousarray(np.asarray(W, np.float32).T).astype(bf)

    wqT, wkT = prep_w(inputs["Wq"]), prep_w(inputs["Wk"])
    wvT, wpT = prep_w(inputs["Wv"]), prep_w(inputs["Wp"])
    bqT = np.ascontiguousarray(np.asarray(inputs["bq"], np.float32).reshape(DC, P).T)
    bkT = np.ascontiguousarray(np.asarray(inputs["bk"], np.float32).reshape(DC, P).T)
    bv_r = np.asarray(inputs["bv"], np.float32).reshape(1, C)
    bp_r = np.asarray(inputs["bp"], np.float32).reshape(1, C)
    pv = (NC * np.arange(P, dtype=np.float32)).reshape(P, 1)

    in_maps = []
    for i in range(NC):
        xTi = np.ascontiguousarray(x[i::NC].T).astype(bf)
        qg = (i + NC * np.arange(QC, dtype=np.float32)).reshape(1, QC)
        in_maps.append({
            "xT": xTi, "wqT": wqT, "wkT": wkT, "wvT": wvT, "wpT": wpT,
            "bqT": bqT, "bkT": bkT, "bv": bv_r, "bp": bp_r,
            "qg": qg, "pv": pv,
        })
    return in_maps


def kernel(x, Wq, bq, Wk, bk, Wv, bv, Wp, bp, _debug=False, _raw=False):
    x = np.asarray(x, dtype=np.float32)
    T, C = x.shape
    NC = NCORES

    nc = _get_nc(T, C, _debug)
    in_maps = build_in_maps(dict(x=x, Wq=Wq, bq=bq, Wk=Wk, bk=bk,
                                 Wv=Wv, bv=bv, Wp=Wp, bp=bp))

    if _raw or _debug:
        res = run_bass_kernel_spmd(nc, in_maps, list(range(NC)))
        if _raw:
            return res
        results = res.results
    else:
        try:
            results = _run_cached(nc, T, C, in_maps)
        except Exception:
            res = run_bass_kernel_spmd(nc, in_maps, list(range(NC)))
            results = res.results
    y = np.empty((T, C), np.float32)
    for i in range(NC):
        y[i::NC] = np.asarray(results[i]["y"]).astype(np.float32)
    return y


_RUNNER_CACHE = {}


def _run_cached(nc, T, C, in_maps):
    """Repeat-call fast path: the sharded PJRT executable and the device-side
    zero output buffers are built once; later calls only transfer inputs."""
    import jax
    from jax.sharding import Mesh, PartitionSpec, NamedSharding
    from jax.experimental.shard_map import shard_map
    import concourse.bass2jax as b2j
    import concourse.mybir as mb

    key = (T, C)
    if key not in _RUNNER_CACHE:
        b2j.install_neuronx_cc_hook()
        partition_name = (nc.partition_id_tensor.name
                          if nc.partition_id_tensor else None)
        in_names, out_names, out_avals, zero_outs = [], [], [], []
        for alloc in nc.m.functions[0].allocations:
            if not isinstance(alloc, mb.MemoryLocationSet):
                continue
            name = alloc.memorylocations[0].name
            if alloc.kind == "ExternalInput":
                if name != partition_name:
                    in_names.append(name)
            elif alloc.kind == "ExternalOutput":
                shape = tuple(alloc.tensor_shape)
                dtype = mb.dt.np(alloc.dtype)
                out_names.append(name)
                out_avals.append(jax.core.ShapedArray(shape, dtype))
                zero_outs.append(np.zeros(shape, dtype))
        n_params = len(in_names)
        all_in = in_names + out_names + ([partition_name] if partition_name else [])

        def _body(*args):
            operands = list(args)
            if partition_name is not None:
                operands.append(b2j.partition_id_tensor())
            return tuple(b2j._bass_exec_p.bind(
                *operands,
                out_avals=tuple(out_avals),
                in_names=tuple(all_in),
                out_names=tuple(out_names),
                lowering_input_output_aliases=(),
                sim_require_finite=True,
                sim_require_nnan=True,
                nc=nc,
            ))

        devices = jax.devices()[:NCORES]
        mesh = Mesh(np.asarray(devices), ("core",))
        n_outs = len(out_names)
        fn = jax.jit(
            shard_map(_body, mesh=mesh,
                      in_specs=(PartitionSpec("core"),) * (n_params + n_outs),
                      out_specs=(PartitionSpec("core"),) * n_outs,
                      check_rep=False),
            keep_unused=True,
        )
        sharding = NamedSharding(mesh, PartitionSpec("core"))
        zeros_dev = [
            jax.device_put(np.zeros((NCORES * z.shape[0], *z.shape[1:]), z.dtype),
                           sharding)
            for z in zero_outs
        ]
        _RUNNER_CACHE[key] = (fn, in_names, out_names, out_avals, zeros_dev, sharding)

    fn, in_names, out_names, out_avals, zeros_dev, sharding = _RUNNER_CACHE[key]
    import jax
    concat_in = [
        jax.device_put(
            np.concatenate([np.asarray(in_maps[c][n]) for c in range(NCORES)],
                           axis=0), sharding)
        for n in in_names
    ]
    outs = fn(*concat_in, *zeros_dev)
    results = []
    for c in range(NCORES):
        results.append({
            name: np.asarray(outs[i]).reshape(NCORES, *out_avals[i].shape)[c]
            for i, name in enumerate(out_names)
        })
    return results
